# revision 7
# baseline (speedup 1.0000x reference)
"""Trainium2 kernel for nn_ApiAdapter_5351529251293 (scatter_memory).

Pipeline: compress/decompress (host, trivial gather/scatter) -> 2 Mamba2
layers over the concatenated 9216-token sequence -> mean -> compress.

The two big matmuls per layer (in_proj [9216,1024]@[1024,4256] and
out_proj [9216,2048]@[2048,1024]) dominate both bytes and FLOPs; they run
on the 8 NeuronCores, data-parallel over the token axis (1152 tokens per
core). The sequential SSM scan, depthwise conv, norms and top-k glue run
on host (cheap, latency-bound).

One generic padded SPMD matmul graph (K=2048, N=4256) is compiled once
and reused for all 4 device launches.
"""

import functools
import numpy as np

# ---- static config (mirrors the reference nn.Module) ----
D_MODEL = 1024
N_LAYERS = 2
D_STATE = 64
D_HEAD = 64
D_INNER = 2048
NHEADS = 32
CONV_DIM = D_INNER + 2 * D_STATE          # 2176
D_IN = 2 * D_INNER + 2 * D_STATE + NHEADS  # 4256
KCONV = 4
PATCH = 32
L = 1024
M = 8
EPS = 1e-5

T_FULL = (1 + M) * L      # 9216
NCORES = 8
T_SH = T_FULL // NCORES   # 1152
KPAD = 2048               # padded contraction dim
NPAD = 4256               # padded output dim
NT = T_SH // 128          # 9 token tiles per core
NK = KPAD // 128          # 16 contraction chunks
N_SIZES = [512] * 8 + [160]   # 4256 = 8*512 + 160

_CACHE = {}


def _build_nc():
    import concourse.bass as bass
    import concourse.tile as tile
    from concourse import bacc, mybir

    f32 = mybir.dt.float32
    nc = bacc.Bacc("TRN2", target_bir_lowering=False, debug=False)
    xT = nc.declare_dram_parameter("xT", [KPAD, T_SH], f32, isOutput=False)
    w = nc.declare_dram_parameter("w", [KPAD, NPAD], f32, isOutput=False)
    out = nc.declare_dram_parameter("out", [T_SH, NPAD], f32, isOutput=True)

    with tile.TileContext(nc) as tc:
        with (
            tc.tile_pool(name="xp", bufs=1) as xp,
            tc.tile_pool(name="wp", bufs=2) as wp,
            tc.tile_pool(name="pp", bufs=4, space="PSUM") as pp,
            tc.tile_pool(name="op", bufs=4) as op,
        ):
            # stage the whole x shard in SBUF: chunk ki lives at cols [ki*T_SH, (ki+1)*T_SH)
            xt = xp.tile([128, NK * T_SH], f32)
            for ki in range(NK):
                nc.sync.dma_start(
                    xt[:, ki * T_SH:(ki + 1) * T_SH],
                    xT[ki * 128:(ki + 1) * 128, :],
                )
            n0 = 0
            for nsz in N_SIZES:
                wt = wp.tile([128, NK * nsz], f32, tag="w")
                for ki in range(NK):
                    nc.sync.dma_start(
                        wt[:, ki * nsz:(ki + 1) * nsz],
                        w[ki * 128:(ki + 1) * 128, n0:n0 + nsz],
                    )
                for mi in range(NT):
                    ps = pp.tile([128, nsz], f32, tag="ps")
                    for ki in range(NK):
                        nc.tensor.matmul(
                            ps[:],
                            xt[:, ki * T_SH + mi * 128: ki * T_SH + mi * 128 + 128],
                            wt[:, ki * nsz:(ki + 1) * nsz],
                            start=(ki == 0),
                            stop=(ki == NK - 1),
                        )
                    ot = op.tile([128, nsz], f32, tag="o")
                    nc.vector.tensor_copy(ot[:], ps[:])
                    nc.sync.dma_start(out[mi * 128:(mi + 1) * 128, n0:n0 + nsz], ot[:])
                n0 += nsz
    nc.compile()
    return nc


def _make_sharded_exec(nc):
    """Build the same jitted shard_map executable run_bass_kernel_spmd uses
    under axon (bass2jax.run_bass_via_pjrt), but cache it so repeat launches
    skip the retrace/recompile."""
    import jax
    import numpy as _np
    from jax.sharding import Mesh, PartitionSpec
    from jax.experimental.shard_map import shard_map
    from concourse import bass2jax, mybir

    bass2jax.install_neuronx_cc_hook()

    partition_name = nc.partition_id_tensor.name if nc.partition_id_tensor else None
    in_names, out_names, out_avals, zero_outs = [], [], [], []
    for alloc in nc.m.functions[0].allocations:
        if not isinstance(alloc, mybir.MemoryLocationSet):
            continue
        name = alloc.memorylocations[0].name
        if alloc.kind == "ExternalInput":
            if name != partition_name:
                in_names.append(name)
        elif alloc.kind == "ExternalOutput":
            shape = tuple(alloc.tensor_shape)
            dtype = mybir.dt.np(alloc.dtype)
            out_avals.append(jax.core.ShapedArray(shape, dtype))
            out_names.append(name)
            zero_outs.append(_np.zeros(shape, dtype))
    n_params = len(in_names)
    n_outs = len(out_avals)
    all_in_names = list(in_names) + list(out_names)
    if partition_name is not None:
        all_in_names.append(partition_name)

    def _body(*args):
        operands = list(args)
        if partition_name is not None:
            operands.append(bass2jax.partition_id_tensor())
        outs = bass2jax._bass_exec_p.bind(
            *operands,
            out_avals=tuple(out_avals),
            in_names=tuple(all_in_names),
            out_names=tuple(out_names),
            lowering_input_output_aliases=(),
            sim_require_finite=True,
            sim_require_nnan=True,
            nc=nc,
        )
        return tuple(outs)

    devices = jax.devices()[:NCORES]
    mesh = Mesh(_np.asarray(devices), ("core",))
    # xT is sharded over cores; w is replicated (uploaded once, not 8x);
    # the out param is a persistent device buffer (my kernel writes every
    # element, so its pre-contents never matter)
    in_specs = tuple(
        PartitionSpec(None) if name == "w" else PartitionSpec("core")
        for name in in_names
    ) + (PartitionSpec("core"),) * n_outs
    out_specs = (PartitionSpec("core"),) * n_outs
    n_params = len(in_names)
    donate = tuple(range(n_params, n_params + n_outs))
    sharded = jax.jit(
        shard_map(_body, mesh=mesh, in_specs=in_specs, out_specs=out_specs,
                  check_rep=False),
        donate_argnums=donate, keep_unused=True,
    )

    from jax.sharding import NamedSharding
    import jax as _jax
    outbufs = [
        _jax.device_put(
            _np.zeros((NCORES * z.shape[0], *z.shape[1:]), z.dtype),
            NamedSharding(mesh, PartitionSpec("core")),
        )
        for z in zero_outs
    ]
    return sharded, in_names, out_names, out_avals, outbufs


def _device_matmul(x, w):
    """y = x @ w on the 8 NeuronCores. x [T_FULL, K], w [K, N]; K<=KPAD, N<=NPAD."""
    if "nc" not in _CACHE:
        _CACHE["nc"] = _build_nc()
        _CACHE["exec"] = _make_sharded_exec(_CACHE["nc"])
    sharded, in_names, out_names, out_avals, outbufs = _CACHE["exec"]

    K = x.shape[1]
    N = w.shape[1]
    xTp = np.zeros((KPAD, T_FULL), np.float32)
    xTp[:K] = x.T
    wp = np.zeros((KPAD, NPAD), np.float32)
    wp[:K, :N] = w
    inputs_by_name = {
        "xT": np.ascontiguousarray(xTp.reshape(KPAD, NCORES, T_SH).transpose(1, 0, 2)
                                   ).reshape(NCORES * KPAD, T_SH),
        "w": wp,
    }
    args = [inputs_by_name[name] for name in in_names]
    out_arrs = sharded(*args, *_CACHE.get("outbufs", outbufs))
    # donated chain: the returned buffers become next call's out params
    _CACHE["outbufs"] = list(out_arrs)
    i = out_names.index("out")
    y = np.asarray(out_arrs[i]).reshape(T_FULL, NPAD)
    return y[:, :N]


# ---------------- host glue (cheap, latency-bound pieces) ----------------

def _rmsnorm(x, w):
    m = np.mean(x * x, axis=-1, keepdims=True, dtype=np.float32)
    return (x / np.sqrt(m + EPS)) * w


def _silu(x):
    return x / (1.0 + np.exp(-x))


def _causal_conv(x, w, b):
    T = x.shape[0]
    xp = np.zeros((T + KCONV - 1, x.shape[1]), np.float32)
    xp[KCONV - 1:] = x
    y = np.zeros_like(x)
    for j in range(KCONV):
        y += xp[j:j + T] * w[:, j]
    return y + b


@functools.lru_cache(maxsize=1)
def _scan_fn():
    import jax
    import jax.numpy as jnp

    cpu = jax.devices("cpu")[0]

    def ssd(dt, A, xh, Bm, Cm):
        def step(h, inputs):
            dt_t, x_t, b_t, c_t = inputs
            dA = jnp.exp(dt_t * A)
            h = h * dA[:, None, None] + (dt_t[:, None] * x_t)[:, :, None] * b_t[None, None, :]
            y = jnp.einsum("hpn,n->hp", h, c_t)
            return h, y

        h0 = jnp.zeros((NHEADS, D_HEAD, D_STATE), xh.dtype)
        _, ys = jax.lax.scan(step, h0, (dt, xh, Bm, Cm))
        return ys

    jitted = jax.jit(ssd)

    def run(*args):
        with jax.default_device(cpu):
            return jitted(*[jax.device_put(np.asarray(a), cpu) for a in args])

    return run


def _ssd(dt, A, xh, Bm, Cm, Dsk):
    ys = np.asarray(_scan_fn()(dt, A, xh, Bm, Cm))
    return ys + Dsk[None, :, None] * xh


def _compress(emb):
    # base is all-zeros, so diff == emb
    ent = np.std(emb, axis=-1, ddof=1)
    order = np.argsort(-ent, kind="stable")[:PATCH]
    idx = np.sort(order)
    return emb[idx], idx


def _mixer(x, in_w, in_b, conv_w, conv_b, dt_bias, A_log, Dsk, norm_w, out_w):
    zxbcdt = _device_matmul(x, in_w) + in_b
    z = zxbcdt[:, :D_INNER]
    xBC = zxbcdt[:, D_INNER:D_INNER + CONV_DIM]
    dt_raw = zxbcdt[:, D_INNER + CONV_DIM:]
    xBC = _silu(_causal_conv(xBC, conv_w, conv_b))
    xh = np.ascontiguousarray(xBC[:, :D_INNER]).reshape(-1, NHEADS, D_HEAD)
    Bm = np.ascontiguousarray(xBC[:, D_INNER:D_INNER + D_STATE])
    Cm = np.ascontiguousarray(xBC[:, D_INNER + D_STATE:])
    dt = np.logaddexp(0.0, dt_raw + dt_bias).astype(np.float32)
    A = -np.exp(A_log)
    y = _ssd(dt, A, xh, Bm, Cm, Dsk).reshape(-1, D_INNER)
    y = _rmsnorm(y * _silu(z), norm_w)
    return _device_matmul(y, out_w)


def kernel(**inputs):
    inp = {k: np.asarray(v) for k, v in inputs.items()}
    query = inp["query"].astype(np.float32)
    memories = inp["memories"].astype(np.float32)

    base = np.zeros_like(query)

    # compress each memory (top-32 rows by std) then scatter back (decompress)
    dec = np.zeros((M, L, D_MODEL), np.float32)
    for m in range(M):
        p, idx = _compress(memories[m])
        dec[m, idx] = p.astype(np.int8).astype(np.float32)

    seq = np.concatenate([query[None], dec], axis=0).reshape(T_FULL, D_MODEL)
    for l in range(N_LAYERS):
        xn = _rmsnorm(seq, inp["prenorm_w"][l])
        seq = seq + _mixer(
            xn, inp["in_w"][l], inp["in_b"][l], inp["conv_w"][l],
            inp["conv_b"][l], inp["dt_bias"][l], inp["A_log"][l],
            inp["D_skip"][l], inp["norm_w"][l], inp["out_w"][l],
        )

    aggregated = seq.reshape(1 + M, L, D_MODEL).mean(axis=0)
    out_patch, out_idx = _compress(aggregated)
    return out_patch.astype(np.int8), out_idx.astype(np.int32), base
